# revision 30
# baseline (speedup 1.0000x reference)
"""LIF spike kernel for Trainium2 (Bass/Tile), data-parallel over batch on 8 cores.

Host layout per core: x_core [C=128, 2, T*2048] f32 — the 4 local batches are
packed as 2 batch-PAIRS (bp); each (bp, t) compute tile is [128, 2048]
(b-even HW columns then b-odd HW columns). This halves the op count vs
per-b tiles: the serial recurrence is 7 links per bp-chain, and the two
chains interleave on the engines.

State is u_t (pre-reset membrane); the hard reset folds into the next step:
  u_t     = select(u_{t-1} <= 1, u_{t-1}, 0) * 0.5 + x_t   (custom DVE op, 1 op/step)
  spike_t = sign(u_t - 1) saturated to uint8               (ACT engine, exact {0,1})

sign->u8 saturates negatives to 0 (HW-verified), so spike == (u > 1) exactly,
including u == 1 (sign(0) = 0). All arithmetic is bit-exact fp32 vs the
reference (mult by 0.5 exact, one rounding add, select exact).

DMA plan: 9 input chunks in consumption order (bp0 first, finer at the start
so the chain starts early) split across the two HWDGE rings — the scalar
engine's ring runs slower while ACT computes, so it carries fewer bytes and
the last-needed chunk. All 5 output slices (finer at the tail) dispatch from
the otherwise-idle sync engine in readiness order; ring FIFOs drain them
after the inputs so input reads keep full bandwidth.

Measured: ~66.5 us HW exec (quiet device) vs 110.6 us baseline; bit-exact
(rel err 0.0). The 8 cores are DMA-bound: 21 MB/core over HBM shared per-chip
by core pairs; noisy-neighbor windows add up to ~10 us.
"""

import numpy as np

import concourse.bacc as bacc
import concourse.mybir as mybir
import concourse.dve_ops as dve_ops
from concourse.dve_ops import DveOp
from concourse.dve_spec import Spec, Src0, Src1, C0, C1, Zero, select, lower, _has_src1
from concourse.dve_uop import DveOpSpec
from concourse.dve_table_gen import dve_ver_for
from concourse.tile import TileContext
from concourse.bass_utils import run_bass_kernel_spmd

B, T, C, H, W = 32, 8, 128, 32, 32
HW = H * W
N_CORES = 8
B_LOC = B // N_CORES
NP = B_LOC // 2  # batch-pairs per core (2)
PW = 2 * HW  # pair width: 2048 columns per (bp, t)
TPW = T * PW  # 16384
TAU = 0.5
THRESH = 1.0

_nc_cache = None


def _register_lif_step():
    name = "LIF_STEP_ANT"
    for op in dve_ops.OPS:
        if op.name == name:
            return op

    def _ref(in0, in1, s0, s1, imm2):
        m = np.where(
            np.asarray(in0, np.float32) <= np.float32(s1), in0, np.float32(0.0)
        ).astype(np.float32)
        return (m * np.float32(s0) + np.asarray(in1, np.float32)).astype(np.float32)

    body = select(Src0 <= C1, Src0, Zero) * C0 + Src1
    spec = Spec(body=body, reference=_ref)
    row = dve_ops._CUSTOM_DVE_ROW_BASE + len(dve_ops.OPS)
    ver = dve_ver_for("TRN2")
    tmp = DveOpSpec(name=name, opcode=row, uops=lower(spec, ver=ver), rd1_en=_has_src1(spec))
    op = DveOp(name, spec, subdim=False, uops_sha={ver: tmp.sha(ver)})
    dve_ops.OPS.append(op)
    dve_ops._SUB_OPCODE_FOR_NAME[name] = row
    dve_ops.CUSTOM_DVE_SPECS[name] = spec
    return op


def build_nc():
    lif_op = _register_lif_step()
    nc = bacc.Bacc("TRN2", target_bir_lowering=False)
    f32 = mybir.dt.float32
    u8 = mybir.dt.uint8
    act = mybir.ActivationFunctionType

    x = nc.dram_tensor("x", [C, NP, TPW], f32, kind="ExternalInput")
    out = nc.dram_tensor("out", [C, NP, TPW], u8, kind="ExternalOutput")

    # (bp, tlo, thi) input chunks in consumption order; bp0's start is finer
    # so the first chain links can begin ~5 us earlier
    # ring per chunk: the scalar engine's ring runs slower while ACT computes,
    # so it gets fewer bytes, and the last-needed chunk (1,6,8) rides it so
    # everything else lands earlier in consumption order
    CHUNKS = [
        ((0, 0, 1), "s"), ((0, 1, 2), "a"), ((0, 2, 4), "s"), ((0, 4, 6), "a"),
        ((0, 6, 8), "s"), ((1, 0, 2), "a"), ((1, 2, 4), "s"), ((1, 4, 6), "s"),
        ((1, 6, 8), "a"),
    ]
    # (bp, tlo, thi) output slices, in readiness order; the tail is finer so
    # the last transfer after the final spike is only 0.5 MB
    OUTS = [(0, 0, 4), (0, 4, 8), (1, 0, 4), (1, 4, 6), (1, 6, 8)]

    with TileContext(nc) as tc:
        with (
            tc.tile_pool(name="xp", bufs=NP) as xp,
            tc.tile_pool(name="up", bufs=3) as up,
            tc.tile_pool(name="sp", bufs=3) as sp,
            tc.tile_pool(name="cp", bufs=1) as cp,
        ):
            negone = cp.tile([C, 1], f32, tag="negone")
            nc.gpsimd.memset(negone[:], -1.0)

            xb = [
                xp.tile([C, TPW], f32, tag="x", name=f"xbp{p}") for p in range(NP)
            ]
            with tc.high_priority():
                for (p, tlo, thi), ring in CHUNKS:
                    eng = nc.sync if ring == "s" else nc.scalar
                    eng.dma_start(
                        out=xb[p][:, tlo * PW : thi * PW],
                        in_=x[:, p, tlo * PW : thi * PW],
                    )

            sbh = {}
            for p in range(NP):
                u_prev = None
                for t in range(T):
                    xs = xb[p][:, t * PW : (t + 1) * PW]
                    if t % 4 == 0:
                        sbh[(p, t // 4)] = sp.tile(
                            [C, 4 * PW], u8, tag="s", name=f"sb{p}_{t // 4}"
                        )
                    sb = sbh[(p, t // 4)]
                    if t == 0:
                        u = xs
                    else:
                        u = up.tile([C, PW], f32, tag="u")
                        nc.vector._custom_dve(
                            lif_op, out=u[:], in0=u_prev[:], in1=xs,
                            s0=TAU, s1=THRESH,
                        )
                    ss = sb[:, (t % 4) * PW : (t % 4 + 1) * PW]
                    if p == NP - 1 and t == T - 1:
                        # final spike on the DVE (tensor_scalar 2x mode, no
                        # cross-engine hop) — shortens the critical tail
                        nc.vector.tensor_scalar(
                            ss, u[:], THRESH, None, mybir.AluOpType.is_gt
                        )
                    else:
                        nc.scalar.activation(
                            ss, u if t == 0 else u[:], act.Sign, bias=negone[:]
                        )
                    u_prev = u
            for p, tlo, thi in OUTS:
                # all outputs on the idle sync engine: they dispatch in
                # readiness order the moment their spikes land, and the ACT
                # stream stays pure compute
                sb = sbh[(p, tlo // 4)]
                lo = (tlo % 4) * PW
                nc.sync.dma_start(
                    out=out[:, p, tlo * PW : thi * PW],
                    in_=sb[:, lo : lo + (thi - tlo) * PW],
                )
    nc.compile()
    return nc


def make_in_maps(x: np.ndarray) -> list[dict]:
    # x [B,T,C,H,W] -> per core [C, NP, T*2048], pair = (b_even | b_odd)
    xs = np.ascontiguousarray(x).reshape(B, T, C, HW)
    maps = []
    for i in range(N_CORES):
        bs = xs[i * B_LOC : (i + 1) * B_LOC]  # [4, T, C, HW]
        pairs = [
            np.concatenate([bs[2 * p], bs[2 * p + 1]], axis=2) for p in range(NP)
        ]  # each [T, C, 2048]
        arr = np.stack(pairs, axis=0)  # [NP, T, C, 2048]
        maps.append(
            {"x": np.ascontiguousarray(arr.transpose(2, 0, 1, 3)).reshape(C, NP, TPW)}
        )
    return maps


def kernel(x: np.ndarray) -> np.ndarray:
    global _nc_cache
    if _nc_cache is None:
        _nc_cache = build_nc()
    res = run_bass_kernel_spmd(_nc_cache, make_in_maps(x), list(range(N_CORES)))
    # out[c, bp, t*2048 + half*1024 + hw] -> [b, t, c, hw], b = 2*bp + half
    parts = []
    for i in range(N_CORES):
        arr = res.results[i]["out"].reshape(C, NP, T, 2, HW)
        parts.append(arr.transpose(1, 3, 2, 0, 4).reshape(B_LOC, T, C, HW))
    full = np.concatenate(parts, axis=0)
    return full.reshape(B, T, C, H, W).astype(np.float32)


# revision 31
# speedup vs baseline: 1.0071x; 1.0071x over previous
"""LIF spike kernel for Trainium2 (Bass/Tile), data-parallel over batch on 8 cores.

Host layout per core: x_core [C=128, 2, T*2048] f32 — the 4 local batches are
packed as 2 batch-PAIRS (bp); each (bp, t) compute tile is [128, 2048]
(b-even HW columns then b-odd HW columns). This halves the op count vs
per-b tiles: the serial recurrence is 7 links per bp-chain, and the two
chains interleave on the engines.

State is u_t (pre-reset membrane); the hard reset folds into the next step:
  u_t     = select(u_{t-1} <= 1, u_{t-1}, 0) * 0.5 + x_t   (custom DVE op, 1 op/step)
  spike_t = sign(u_t - 1) saturated to uint8               (ACT engine, exact {0,1})

sign->u8 saturates negatives to 0 (HW-verified), so spike == (u > 1) exactly,
including u == 1 (sign(0) = 0). All arithmetic is bit-exact fp32 vs the
reference (mult by 0.5 exact, one rounding add, select exact).

DMA plan: 9 input chunks in consumption order (bp0 first, finer at the start
so the chain starts early) split across the two HWDGE rings — the scalar
engine's ring runs slower while ACT computes, so it carries fewer bytes and
the last-needed chunk. All 5 output slices (finer at the tail) dispatch from
the otherwise-idle sync engine in readiness order; ring FIFOs drain them
after the inputs so input reads keep full bandwidth.

Measured: ~66.5 us HW exec (quiet device) vs 110.6 us baseline; bit-exact
(rel err 0.0). The 8 cores are DMA-bound: 21 MB/core over HBM shared per-chip
by core pairs; noisy-neighbor windows add up to ~10 us.
"""

import numpy as np

import concourse.bacc as bacc
import concourse.mybir as mybir
import concourse.dve_ops as dve_ops
from concourse.dve_ops import DveOp
from concourse.dve_spec import Spec, Src0, Src1, C0, C1, Zero, select, lower, _has_src1
from concourse.dve_uop import DveOpSpec
from concourse.dve_table_gen import dve_ver_for
from concourse.tile import TileContext
from concourse.bass_utils import run_bass_kernel_spmd

B, T, C, H, W = 32, 8, 128, 32, 32
HW = H * W
N_CORES = 8
B_LOC = B // N_CORES
NP = B_LOC // 2  # batch-pairs per core (2)
PW = 2 * HW  # pair width: 2048 columns per (bp, t)
TPW = T * PW  # 16384
TAU = 0.5
THRESH = 1.0

_nc_cache = None


def _register_lif_step():
    name = "LIF_STEP_ANT"
    for op in dve_ops.OPS:
        if op.name == name:
            return op

    def _ref(in0, in1, s0, s1, imm2):
        m = np.where(
            np.asarray(in0, np.float32) <= np.float32(s1), in0, np.float32(0.0)
        ).astype(np.float32)
        return (m * np.float32(s0) + np.asarray(in1, np.float32)).astype(np.float32)

    body = select(Src0 <= C1, Src0, Zero) * C0 + Src1
    spec = Spec(body=body, reference=_ref)
    row = dve_ops._CUSTOM_DVE_ROW_BASE + len(dve_ops.OPS)
    ver = dve_ver_for("TRN2")
    tmp = DveOpSpec(name=name, opcode=row, uops=lower(spec, ver=ver), rd1_en=_has_src1(spec))
    op = DveOp(name, spec, subdim=False, uops_sha={ver: tmp.sha(ver)})
    dve_ops.OPS.append(op)
    dve_ops._SUB_OPCODE_FOR_NAME[name] = row
    dve_ops.CUSTOM_DVE_SPECS[name] = spec
    return op


def build_nc():
    lif_op = _register_lif_step()
    nc = bacc.Bacc("TRN2", target_bir_lowering=False)
    f32 = mybir.dt.float32
    u8 = mybir.dt.uint8
    act = mybir.ActivationFunctionType

    x = nc.dram_tensor("x", [C, NP, TPW], f32, kind="ExternalInput")
    out = nc.dram_tensor("out", [C, NP, TPW], u8, kind="ExternalOutput")

    # (bp, tlo, thi) input chunks in consumption order; bp0's start is finer
    # so the first chain links can begin ~5 us earlier
    # ring per chunk: the scalar engine's ring runs slower while ACT computes,
    # so it gets fewer bytes, and the last-needed chunk (1,6,8) rides it so
    # everything else lands earlier in consumption order
    CHUNKS = [
        ((0, 0, 1), "s"), ((0, 1, 2), "a"), ((0, 2, 4), "s"), ((0, 4, 6), "a"),
        ((0, 6, 8), "s"), ((1, 0, 2), "a"), ((1, 2, 4), "s"), ((1, 4, 6), "s"),
        ((1, 6, 8), "a"),
    ]
    # (bp, tlo, thi) output slices, in readiness order; the tail is finer so
    # the last transfer after the final spike is only 0.25 MB (t6's slice
    # ships while t7 still computes)
    OUTS = [(0, 0, 4), (0, 4, 8), (1, 0, 4), (1, 4, 6), (1, 6, 7), (1, 7, 8)]

    with TileContext(nc) as tc:
        with (
            tc.tile_pool(name="xp", bufs=NP) as xp,
            tc.tile_pool(name="up", bufs=3) as up,
            tc.tile_pool(name="sp", bufs=3) as sp,
            tc.tile_pool(name="cp", bufs=1) as cp,
        ):
            negone = cp.tile([C, 1], f32, tag="negone")
            nc.gpsimd.memset(negone[:], -1.0)

            xb = [
                xp.tile([C, TPW], f32, tag="x", name=f"xbp{p}") for p in range(NP)
            ]
            with tc.high_priority():
                for (p, tlo, thi), ring in CHUNKS:
                    eng = nc.sync if ring == "s" else nc.scalar
                    eng.dma_start(
                        out=xb[p][:, tlo * PW : thi * PW],
                        in_=x[:, p, tlo * PW : thi * PW],
                    )

            sbh = {}
            for p in range(NP):
                u_prev = None
                for t in range(T):
                    xs = xb[p][:, t * PW : (t + 1) * PW]
                    if t % 4 == 0:
                        sbh[(p, t // 4)] = sp.tile(
                            [C, 4 * PW], u8, tag="s", name=f"sb{p}_{t // 4}"
                        )
                    sb = sbh[(p, t // 4)]
                    if t == 0:
                        u = xs
                    else:
                        u = up.tile([C, PW], f32, tag="u")
                        nc.vector._custom_dve(
                            lif_op, out=u[:], in0=u_prev[:], in1=xs,
                            s0=TAU, s1=THRESH,
                        )
                    ss = sb[:, (t % 4) * PW : (t % 4 + 1) * PW]
                    if p == NP - 1 and t == T - 1:
                        # final spike on the DVE (tensor_scalar 2x mode, no
                        # cross-engine hop) — shortens the critical tail
                        nc.vector.tensor_scalar(
                            ss, u[:], THRESH, None, mybir.AluOpType.is_gt
                        )
                    else:
                        nc.scalar.activation(
                            ss, u if t == 0 else u[:], act.Sign, bias=negone[:]
                        )
                    u_prev = u
            for p, tlo, thi in OUTS:
                # all outputs on the idle sync engine: they dispatch in
                # readiness order the moment their spikes land, and the ACT
                # stream stays pure compute
                sb = sbh[(p, tlo // 4)]
                lo = (tlo % 4) * PW
                nc.sync.dma_start(
                    out=out[:, p, tlo * PW : thi * PW],
                    in_=sb[:, lo : lo + (thi - tlo) * PW],
                )
    nc.compile()
    return nc


def make_in_maps(x: np.ndarray) -> list[dict]:
    # x [B,T,C,H,W] -> per core [C, NP, T*2048], pair = (b_even | b_odd)
    xs = np.ascontiguousarray(x).reshape(B, T, C, HW)
    maps = []
    for i in range(N_CORES):
        bs = xs[i * B_LOC : (i + 1) * B_LOC]  # [4, T, C, HW]
        pairs = [
            np.concatenate([bs[2 * p], bs[2 * p + 1]], axis=2) for p in range(NP)
        ]  # each [T, C, 2048]
        arr = np.stack(pairs, axis=0)  # [NP, T, C, 2048]
        maps.append(
            {"x": np.ascontiguousarray(arr.transpose(2, 0, 1, 3)).reshape(C, NP, TPW)}
        )
    return maps


def kernel(x: np.ndarray) -> np.ndarray:
    global _nc_cache
    if _nc_cache is None:
        _nc_cache = build_nc()
    res = run_bass_kernel_spmd(_nc_cache, make_in_maps(x), list(range(N_CORES)))
    # out[c, bp, t*2048 + half*1024 + hw] -> [b, t, c, hw], b = 2*bp + half
    parts = []
    for i in range(N_CORES):
        arr = res.results[i]["out"].reshape(C, NP, T, 2, HW)
        parts.append(arr.transpose(1, 3, 2, 0, 4).reshape(B_LOC, T, C, HW))
    full = np.concatenate(parts, axis=0)
    return full.reshape(B, T, C, H, W).astype(np.float32)


# revision 32
# speedup vs baseline: 1.0941x; 1.0863x over previous
"""LIF spike kernel for Trainium2 (Bass/Tile), data-parallel over batch on 8 cores.

Host layout per core: x_core [C=128, 2, T*2048] f32 — the 4 local batches are
packed as 2 batch-PAIRS (bp); each (bp, t) compute tile is [128, 2048]
(b-even HW columns then b-odd HW columns). This halves the op count vs
per-b tiles: the serial recurrence is 7 links per bp-chain, and the two
chains interleave on the engines.

State is u_t (pre-reset membrane); the hard reset folds into the next step:
  u_t     = select(u_{t-1} <= 1, u_{t-1}, 0) * 0.5 + x_t   (custom DVE op, 1 op/step)
  spike_t = sign(u_t - 1) saturated to uint8               (ACT engine, exact {0,1})

sign->u8 saturates negatives to 0 (HW-verified), so spike == (u > 1) exactly,
including u == 1 (sign(0) = 0). All arithmetic is bit-exact fp32 vs the
reference (mult by 0.5 exact, one rounding add, select exact).

DMA plan: 9 input chunks in consumption order (bp0 first, finer at the start
so the chain starts early) split across the two HWDGE rings — the scalar
engine's ring runs slower while ACT computes, so it carries fewer bytes and
the last-needed chunk. All 5 output slices (finer at the tail) dispatch from
the otherwise-idle sync engine in readiness order; ring FIFOs drain them
after the inputs so input reads keep full bandwidth.

Measured: ~66.5 us HW exec (quiet device) vs 110.6 us baseline; bit-exact
(rel err 0.0). The 8 cores are DMA-bound: 21 MB/core over HBM shared per-chip
by core pairs; noisy-neighbor windows add up to ~10 us.
"""

import numpy as np

import concourse.bacc as bacc
import concourse.mybir as mybir
import concourse.dve_ops as dve_ops
from concourse.dve_ops import DveOp
from concourse.dve_spec import Spec, Src0, Src1, C0, C1, Zero, select, lower, _has_src1
from concourse.dve_uop import DveOpSpec
from concourse.dve_table_gen import dve_ver_for
from concourse.tile import TileContext
from concourse.bass_utils import run_bass_kernel_spmd

B, T, C, H, W = 32, 8, 128, 32, 32
HW = H * W
N_CORES = 8
B_LOC = B // N_CORES
NP = B_LOC // 2  # batch-pairs per core (2)
PW = 2 * HW  # pair width: 2048 columns per (bp, t)
TPW = T * PW  # 16384
TAU = 0.5
THRESH = 1.0

_nc_cache = None


def _register_lif_step():
    name = "LIF_STEP_ANT"
    for op in dve_ops.OPS:
        if op.name == name:
            return op

    def _ref(in0, in1, s0, s1, imm2):
        m = np.where(
            np.asarray(in0, np.float32) <= np.float32(s1), in0, np.float32(0.0)
        ).astype(np.float32)
        return (m * np.float32(s0) + np.asarray(in1, np.float32)).astype(np.float32)

    body = select(Src0 <= C1, Src0, Zero) * C0 + Src1
    spec = Spec(body=body, reference=_ref)
    row = dve_ops._CUSTOM_DVE_ROW_BASE + len(dve_ops.OPS)
    ver = dve_ver_for("TRN2")
    tmp = DveOpSpec(name=name, opcode=row, uops=lower(spec, ver=ver), rd1_en=_has_src1(spec))
    op = DveOp(name, spec, subdim=False, uops_sha={ver: tmp.sha(ver)})
    dve_ops.OPS.append(op)
    dve_ops._SUB_OPCODE_FOR_NAME[name] = row
    dve_ops.CUSTOM_DVE_SPECS[name] = spec
    return op


def build_nc():
    lif_op = _register_lif_step()
    nc = bacc.Bacc("TRN2", target_bir_lowering=False)
    f32 = mybir.dt.float32
    u8 = mybir.dt.uint8
    act = mybir.ActivationFunctionType

    x = nc.dram_tensor("x", [C, NP, TPW], f32, kind="ExternalInput")
    out = nc.dram_tensor("out", [C, NP, TPW], u8, kind="ExternalOutput")

    # (bp, tlo, thi) input chunks in consumption order; bp0's start is finer
    # so the first chain links can begin ~5 us earlier
    # ring per chunk: the scalar engine's ring runs slower while ACT computes,
    # so it gets fewer bytes, and the last-needed chunk (1,6,8) rides it so
    # everything else lands earlier in consumption order
    CHUNKS = [
        ((0, 0, 1), "s"), ((0, 1, 2), "a"), ((0, 2, 4), "s"), ((0, 4, 6), "a"),
        ((0, 6, 8), "s"), ((1, 0, 2), "a"), ((1, 2, 4), "s"), ((1, 4, 6), "s"),
        ((1, 6, 7), "a"), ((1, 7, 8), "a"),
    ]
    # (bp, tlo, thi) output slices, in readiness order; the tail is finer so
    # the last transfer after the final spike is only 0.25 MB (t6's slice
    # ships while t7 still computes)
    OUTS = [(0, 0, 4), (0, 4, 8), (1, 0, 4), (1, 4, 6), (1, 6, 7), (1, 7, 8)]

    with TileContext(nc) as tc:
        with (
            tc.tile_pool(name="xp", bufs=NP) as xp,
            tc.tile_pool(name="up", bufs=3) as up,
            tc.tile_pool(name="sp", bufs=3) as sp,
            tc.tile_pool(name="cp", bufs=1) as cp,
        ):
            negone = cp.tile([C, 1], f32, tag="negone")
            nc.gpsimd.memset(negone[:], -1.0)

            xb = [
                xp.tile([C, TPW], f32, tag="x", name=f"xbp{p}") for p in range(NP)
            ]
            with tc.high_priority():
                for (p, tlo, thi), ring in CHUNKS:
                    eng = nc.sync if ring == "s" else nc.scalar
                    eng.dma_start(
                        out=xb[p][:, tlo * PW : thi * PW],
                        in_=x[:, p, tlo * PW : thi * PW],
                    )

            sbh = {}
            for p in range(NP):
                u_prev = None
                for t in range(T):
                    xs = xb[p][:, t * PW : (t + 1) * PW]
                    if t % 4 == 0:
                        sbh[(p, t // 4)] = sp.tile(
                            [C, 4 * PW], u8, tag="s", name=f"sb{p}_{t // 4}"
                        )
                    sb = sbh[(p, t // 4)]
                    if t == 0:
                        u = xs
                    else:
                        u = up.tile([C, PW], f32, tag="u")
                        nc.vector._custom_dve(
                            lif_op, out=u[:], in0=u_prev[:], in1=xs,
                            s0=TAU, s1=THRESH,
                        )
                    ss = sb[:, (t % 4) * PW : (t % 4 + 1) * PW]
                    if p == NP - 1 and t == T - 1:
                        # final spike on the DVE (tensor_scalar 2x mode, no
                        # cross-engine hop) — shortens the critical tail
                        nc.vector.tensor_scalar(
                            ss, u[:], THRESH, None, mybir.AluOpType.is_gt
                        )
                    else:
                        nc.scalar.activation(
                            ss, u if t == 0 else u[:], act.Sign, bias=negone[:]
                        )
                    u_prev = u
            for p, tlo, thi in OUTS:
                # all outputs on the idle sync engine: they dispatch in
                # readiness order the moment their spikes land, and the ACT
                # stream stays pure compute
                sb = sbh[(p, tlo // 4)]
                lo = (tlo % 4) * PW
                nc.sync.dma_start(
                    out=out[:, p, tlo * PW : thi * PW],
                    in_=sb[:, lo : lo + (thi - tlo) * PW],
                )
    nc.compile()
    return nc


def make_in_maps(x: np.ndarray) -> list[dict]:
    # x [B,T,C,H,W] -> per core [C, NP, T*2048], pair = (b_even | b_odd)
    xs = np.ascontiguousarray(x).reshape(B, T, C, HW)
    maps = []
    for i in range(N_CORES):
        bs = xs[i * B_LOC : (i + 1) * B_LOC]  # [4, T, C, HW]
        pairs = [
            np.concatenate([bs[2 * p], bs[2 * p + 1]], axis=2) for p in range(NP)
        ]  # each [T, C, 2048]
        arr = np.stack(pairs, axis=0)  # [NP, T, C, 2048]
        maps.append(
            {"x": np.ascontiguousarray(arr.transpose(2, 0, 1, 3)).reshape(C, NP, TPW)}
        )
    return maps


def kernel(x: np.ndarray) -> np.ndarray:
    global _nc_cache
    if _nc_cache is None:
        _nc_cache = build_nc()
    res = run_bass_kernel_spmd(_nc_cache, make_in_maps(x), list(range(N_CORES)))
    # out[c, bp, t*2048 + half*1024 + hw] -> [b, t, c, hw], b = 2*bp + half
    parts = []
    for i in range(N_CORES):
        arr = res.results[i]["out"].reshape(C, NP, T, 2, HW)
        parts.append(arr.transpose(1, 3, 2, 0, 4).reshape(B_LOC, T, C, HW))
    full = np.concatenate(parts, axis=0)
    return full.reshape(B, T, C, H, W).astype(np.float32)


# revision 33
# speedup vs baseline: 1.1562x; 1.0567x over previous
"""LIF spike kernel for Trainium2 (Bass/Tile), data-parallel over batch on 8 cores.

Host layout per core: x_core [C=128, 2, T*2048] f32 — the 4 local batches are
packed as 2 batch-PAIRS (bp); each (bp, t) compute tile is [128, 2048]
(b-even HW columns then b-odd HW columns). This halves the op count vs
per-b tiles: the serial recurrence is 7 links per bp-chain, and the two
chains interleave on the engines.

State is u_t (pre-reset membrane); the hard reset folds into the next step:
  u_t     = select(u_{t-1} <= 1, u_{t-1}, 0) * 0.5 + x_t   (custom DVE op, 1 op/step)
  spike_t = sign(u_t - 1) saturated to uint8               (ACT engine, exact {0,1})

sign->u8 saturates negatives to 0 (HW-verified), so spike == (u > 1) exactly,
including u == 1 (sign(0) = 0). All arithmetic is bit-exact fp32 vs the
reference (mult by 0.5 exact, one rounding add, select exact).

DMA plan: 9 input chunks in consumption order (bp0 first, finer at the start
so the chain starts early) split across the two HWDGE rings — the scalar
engine's ring runs slower while ACT computes, so it carries fewer bytes and
the last-needed chunk. All 5 output slices (finer at the tail) dispatch from
the otherwise-idle sync engine in readiness order; ring FIFOs drain them
after the inputs so input reads keep full bandwidth.

Measured: ~66.5 us HW exec (quiet device) vs 110.6 us baseline; bit-exact
(rel err 0.0). The 8 cores are DMA-bound: 21 MB/core over HBM shared per-chip
by core pairs; noisy-neighbor windows add up to ~10 us.
"""

import numpy as np

import concourse.bacc as bacc
import concourse.mybir as mybir
import concourse.dve_ops as dve_ops
from concourse.dve_ops import DveOp
from concourse.dve_spec import Spec, Src0, Src1, C0, C1, Zero, select, lower, _has_src1
from concourse.dve_uop import DveOpSpec
from concourse.dve_table_gen import dve_ver_for
from concourse.tile import TileContext
from concourse.bass_utils import run_bass_kernel_spmd

B, T, C, H, W = 32, 8, 128, 32, 32
HW = H * W
N_CORES = 8
B_LOC = B // N_CORES
NP = B_LOC // 2  # batch-pairs per core (2)
PW = 2 * HW  # pair width: 2048 columns per (bp, t)
TPW = T * PW  # 16384
TAU = 0.5
THRESH = 1.0

_nc_cache = None


def _register_lif_step():
    name = "LIF_STEP_ANT"
    for op in dve_ops.OPS:
        if op.name == name:
            return op

    def _ref(in0, in1, s0, s1, imm2):
        m = np.where(
            np.asarray(in0, np.float32) <= np.float32(s1), in0, np.float32(0.0)
        ).astype(np.float32)
        return (m * np.float32(s0) + np.asarray(in1, np.float32)).astype(np.float32)

    body = select(Src0 <= C1, Src0, Zero) * C0 + Src1
    spec = Spec(body=body, reference=_ref)
    row = dve_ops._CUSTOM_DVE_ROW_BASE + len(dve_ops.OPS)
    ver = dve_ver_for("TRN2")
    tmp = DveOpSpec(name=name, opcode=row, uops=lower(spec, ver=ver), rd1_en=_has_src1(spec))
    op = DveOp(name, spec, subdim=False, uops_sha={ver: tmp.sha(ver)})
    dve_ops.OPS.append(op)
    dve_ops._SUB_OPCODE_FOR_NAME[name] = row
    dve_ops.CUSTOM_DVE_SPECS[name] = spec
    return op


def build_nc():
    lif_op = _register_lif_step()
    nc = bacc.Bacc("TRN2", target_bir_lowering=False)
    f32 = mybir.dt.float32
    u8 = mybir.dt.uint8
    act = mybir.ActivationFunctionType

    x = nc.dram_tensor("x", [C, NP, TPW], f32, kind="ExternalInput")
    out = nc.dram_tensor("out", [C, NP, TPW], u8, kind="ExternalOutput")

    # (bp, tlo, thi) input chunks in consumption order; bp0's start is finer
    # so the first chain links can begin ~5 us earlier
    # ring per chunk: the scalar engine's ring runs slower while ACT computes,
    # so it gets fewer bytes, and the last-needed chunk (1,6,8) rides it so
    # everything else lands earlier in consumption order
    CHUNKS = [
        ((0, 0, 1), "s"), ((0, 1, 2), "a"), ((0, 2, 4), "s"), ((0, 4, 6), "a"),
        ((0, 6, 8), "s"), ((1, 0, 2), "a"), ((1, 2, 4), "s"), ((1, 4, 6), "s"),
        ((1, 6, 8), "a"),
    ]
    # (bp, tlo, thi) output slices, in readiness order; the tail is finer so
    # the last transfer after the final spike is only 0.25 MB (t6's slice
    # ships while t7 still computes)
    OUTS = [(0, 0, 4), (0, 4, 8), (1, 0, 4), (1, 4, 6), (1, 6, 7), (1, 7, 8)]

    with TileContext(nc) as tc:
        with (
            tc.tile_pool(name="xp", bufs=NP) as xp,
            tc.tile_pool(name="up", bufs=3) as up,
            tc.tile_pool(name="sp", bufs=3) as sp,
            tc.tile_pool(name="cp", bufs=1) as cp,
        ):
            negone = cp.tile([C, 1], f32, tag="negone")
            nc.gpsimd.memset(negone[:], -1.0)

            xb = [
                xp.tile([C, TPW], f32, tag="x", name=f"xbp{p}") for p in range(NP)
            ]
            with tc.high_priority():
                for (p, tlo, thi), ring in CHUNKS:
                    eng = nc.sync if ring == "s" else nc.scalar
                    eng.dma_start(
                        out=xb[p][:, tlo * PW : thi * PW],
                        in_=x[:, p, tlo * PW : thi * PW],
                    )

            sbh = {}
            for p in range(NP):
                u_prev = None
                for t in range(T):
                    xs = xb[p][:, t * PW : (t + 1) * PW]
                    if t % 4 == 0:
                        sbh[(p, t // 4)] = sp.tile(
                            [C, 4 * PW], u8, tag="s", name=f"sb{p}_{t // 4}"
                        )
                    sb = sbh[(p, t // 4)]
                    if t == 0:
                        u = xs
                    else:
                        u = up.tile([C, PW], f32, tag="u")
                        nc.vector._custom_dve(
                            lif_op, out=u[:], in0=u_prev[:], in1=xs,
                            s0=TAU, s1=THRESH,
                        )
                    ss = sb[:, (t % 4) * PW : (t % 4 + 1) * PW]
                    if p == NP - 1 and t == T - 1:
                        # final spike on the DVE (tensor_scalar 2x mode, no
                        # cross-engine hop) — shortens the critical tail
                        nc.vector.tensor_scalar(
                            ss, u[:], THRESH, None, mybir.AluOpType.is_gt
                        )
                    else:
                        nc.scalar.activation(
                            ss, u if t == 0 else u[:], act.Sign, bias=negone[:]
                        )
                    u_prev = u
            for p, tlo, thi in OUTS:
                # all outputs on the idle sync engine: they dispatch in
                # readiness order the moment their spikes land, and the ACT
                # stream stays pure compute
                sb = sbh[(p, tlo // 4)]
                lo = (tlo % 4) * PW
                nc.sync.dma_start(
                    out=out[:, p, tlo * PW : thi * PW],
                    in_=sb[:, lo : lo + (thi - tlo) * PW],
                )
    nc.compile()
    return nc


def make_in_maps(x: np.ndarray) -> list[dict]:
    # x [B,T,C,H,W] -> per core [C, NP, T*2048], pair = (b_even | b_odd)
    xs = np.ascontiguousarray(x).reshape(B, T, C, HW)
    maps = []
    for i in range(N_CORES):
        bs = xs[i * B_LOC : (i + 1) * B_LOC]  # [4, T, C, HW]
        pairs = [
            np.concatenate([bs[2 * p], bs[2 * p + 1]], axis=2) for p in range(NP)
        ]  # each [T, C, 2048]
        arr = np.stack(pairs, axis=0)  # [NP, T, C, 2048]
        maps.append(
            {"x": np.ascontiguousarray(arr.transpose(2, 0, 1, 3)).reshape(C, NP, TPW)}
        )
    return maps


def kernel(x: np.ndarray) -> np.ndarray:
    global _nc_cache
    if _nc_cache is None:
        _nc_cache = build_nc()
    res = run_bass_kernel_spmd(_nc_cache, make_in_maps(x), list(range(N_CORES)))
    # out[c, bp, t*2048 + half*1024 + hw] -> [b, t, c, hw], b = 2*bp + half
    parts = []
    for i in range(N_CORES):
        arr = res.results[i]["out"].reshape(C, NP, T, 2, HW)
        parts.append(arr.transpose(1, 3, 2, 0, 4).reshape(B_LOC, T, C, HW))
    full = np.concatenate(parts, axis=0)
    return full.reshape(B, T, C, H, W).astype(np.float32)
